# revision 7
# baseline (speedup 1.0000x reference)
"""v3: layout-B NodeAttention kernel, optimized.

Changes vs v2 baseline:
- Main matmul streams float32r (1 cyc/row at free>=256 vs 4 for fp32).
- ELU path: one DVE op computes v = elu(z)+1 = max(min(e,1), z') directly
  (LN stats are shift-invariant; the +1 cancels exactly in alpha/var).
- Single activation table: the whole program uses only Exp/Tanh/Copy
  (all in act table 0, exp_and_others) -> one table load total.
  rstd comes from a quake-seed Newton rsqrt on gpsimd; the sigmoid is
  tanh-based: sigmoid(a) = 0.5 + 0.5*tanh(a/2).
- sq = v^2 runs on DVE in fp16 (2x/4x mode) instead of gpsimd.
- Host permutes xt columns so each finalize group's gate slab maps to a
  contiguous per-partition j-range of the natural x layout: gating DMAs
  move 5KB-contiguous runs instead of 128B node gathers.
- v-op split DVE/Pool by tile parity to balance engines.
"""

import sys

for _p in ("/opt/trn_rl_repo", "/root/.axon_site/_ro/trn_rl_repo"):
    if _p not in sys.path:
        sys.path.insert(0, _p)

import contextlib
import os

import numpy as np

import concourse.bacc as bacc
import concourse.bass as bass
import concourse.tile as tile
from concourse import mybir
from concourse.bass_utils import run_bass_kernel_spmd

B = 32
N_NODES = 8192
CPN = 32
HID = 256
LN_EPS = 1e-5

NCORES = 8
BPC = B // NCORES
J = BPC * N_NODES            # 32768 node-instances per core
NTILE = 512                  # nodes per matmul tile
UMAX = 15                    # stat-shift slots per stats bank (2*15 <= 32)

F32 = mybir.dt.float32
F32R = mybir.dt.float32r
F16 = mybir.dt.float16
I32 = mybir.dt.int32

AT = mybir.ActivationFunctionType
OP = mybir.AluOpType

# engine split knobs: per v/sq-op index (0..63), run on Pool (gpsimd) when
# (idx % MOD) < POOL.
# NOTE: gpsimd cannot access PSUM on hardware, so the v-op (which reads the
# PSUM matmul output) must stay on DVE.
# per-op engine pattern for sq: P=Pool, A=Act, D=DVE; interleaved so no
# engine sees a burst it cannot drain within a slot
SQ_PATTERN = os.environ.get("K3_SQ_PATTERN", "PAPDPAPD")
GATE_ENGINE = os.environ.get("K3_GATE_ENGINE", "gpsimd")
COPY_ENGINE = os.environ.get("K3_COPY_ENGINE", "scalar")
NEWTON_ITERS = int(os.environ.get("K3_NEWTON", "1"))
# NOTE: only plain TensorTensor/TensorCopy lower to the Pool engine on HW;
# TensorScalar ops do not, so the Newton chain stays on DVE.
NEWTON_ENGINE = os.environ.get("K3_NEWTON_ENGINE", "vector")
STAT_LAG = int(os.environ.get("K3_STAT_LAG", "3"))
FIN_DEFER = int(os.environ.get("K3_FIN_DEFER", "0"))
MID_BUFS = int(os.environ.get("K3_MID_BUFS", "6"))
ZP_BUFS = int(os.environ.get("K3_ZP_BUFS", "3"))
SP_BUFS = int(os.environ.get("K3_SP_BUFS", "1"))
TP_BUFS = int(os.environ.get("K3_TP_BUFS", "1"))
QUAKE = 0x5F3759DF


def _stats_ap(trans, ucnt, j, s, step):
    """View of transposed stats [128, u=ucnt, k=4] for half-block j, slot s.

    Stats for slot u live at row step*u + s of quadrant j (s1/sw quadrants
    use step=2 with s in {0,1}; the sq quadrants use step=1, s=0).
    """
    v = trans.rearrange("p (k j m) -> p k j m", k=4, j=4)
    v = v[:, :, j, s:s + step * ucnt]
    v = v.rearrange("p k (u s) -> p k u s", s=step)[:, :, :, 0]
    return v.rearrange("p k u -> p u k")


def _slab_ap(dram, g, ucnt):
    """Contiguous-slab DRAM view [p=128, u=ucnt, k=4, c=32] for group g.

    Node (p, t, k) lives at index p*256 + t*4 + k of the natural [J, CPN]
    layout (t = g*UMAX + u).  Per partition p the group's nodes are the
    contiguous j-range [g*4*UMAX, g*4*UMAX + 4*ucnt).
    """
    off = g * (4 * UMAX) * CPN
    return bass.AP(tensor=dram.tensor, offset=dram.offset + off,
                   ap=[[(J // 128) * CPN, 128], [4 * CPN, ucnt],
                       [CPN, 4], [1, CPN]])


def _build_program(W1, b1, w2p, s_w2, c_a, j=J, num_devices=NCORES):
    J_, NT = j, j // NTILE
    nc = bacc.Bacc("TRN2", target_bir_lowering=False, debug=False,
                   num_devices=num_devices)

    # xt carries W1a (bias row baked) in its last HID columns so the first
    # tile's DMA also delivers the weights (one less warmup DMA round)
    xt_d = nc.dram_tensor("xt", [CPN + 1, J_ + HID], F32R,
                          kind="ExternalInput").ap()
    xn_d = nc.dram_tensor("xn", [J_, CPN], F32, kind="ExternalInput").ap()
    # blob cols: [0,32)/[32,64)/[64,96) the three f16 stat matrices
    # (f32-aliased); [96,224) identity
    blob_d = nc.dram_tensor("blob", [128, 224], F32,
                            kind="ExternalInput").ap()
    out_d = nc.dram_tensor("out", [J_, CPN], F32, kind="ExternalOutput").ap()

    with tile.TileContext(nc) as tc, contextlib.ExitStack() as ctx:
        const = ctx.enter_context(tc.tile_pool(name="const", bufs=1))
        xt_p = ctx.enter_context(tc.tile_pool(name="xtp", bufs=4))
        zp = ctx.enter_context(tc.tile_pool(name="zp", bufs=ZP_BUFS,
                                            space="PSUM"))
        sp = ctx.enter_context(tc.tile_pool(name="sp", bufs=SP_BUFS,
                                            space="PSUM"))
        tp = ctx.enter_context(tc.tile_pool(name="tp", bufs=TP_BUFS,
                                            space="PSUM"))
        mid = ctx.enter_context(tc.tile_pool(name="mid", bufs=MID_BUFS))
        tl = ctx.enter_context(tc.tile_pool(name="tl", bufs=2))
        gp = ctx.enter_context(tc.tile_pool(name="gp", bufs=3))

        # xt layout: [tiles 0-1 | w1a | tiles 2..]; one DMA brings the
        # first two node tiles and the weights together
        w1a_t = const.tile([CPN + 1, 2 * NTILE + HID], F32R)
        nc.sync.dma_start(
            out=w1a_t[:],
            in_=bass.AP(tensor=xt_d.tensor, offset=xt_d.offset,
                        ap=[[J_ + HID, CPN + 1], [1, 2 * NTILE + HID]]))
        w1a_s = w1a_t[:, 2 * NTILE:]
        # blob (stat matrices + identity) is first needed STAT_LAG slots in;
        # emitted inside the first loop iteration so xt0 wins the DMA queue
        blob_s = const.tile([128, 224], F32)
        sa_s = blob_s[:, 0:32].bitcast(F16)
        sb_s = blob_s[:, 32:64].bitcast(F16)
        sq_s = blob_s[:, 64:96].bitcast(F16)
        ident_s = blob_s[:, 96:224]
        neg1_s = const.tile([128, 1], F32)
        nc.vector.memset(neg1_s, -1.0)
        m05_s = const.tile([128, 1], F32)
        nc.vector.memset(m05_s, -0.5)
        hca_s = const.tile([128, 1], F32)
        nc.vector.memset(hca_s, 0.5 * c_a)

        stats_ps = None
        vs = {}
        sqs = {}

        def finalize_group(g, ucnt):
            if COPY_ENGINE == "scalar":
                cp_eng = nc.scalar.copy
            else:
                def cp_eng(o, i):
                    nc.gpsimd.tensor_copy(out=o, in_=i)
            scopy = mid.tile([128, 512], F32, tag="scopy")
            # split the PSUM read-out across two engines to shorten the
            # group-boundary stall (sp has a single buffer)
            cp_eng(scopy[:, 0:256], stats_ps[:, 0:256])
            nc.vector.tensor_copy(out=scopy[:, 256:512],
                                  in_=stats_ps[:, 256:512])
            trans_ps = tp.tile([128, 512], F32, tag="trans")
            for k in range(4):
                nc.tensor.transpose(trans_ps[:, 128 * k:128 * (k + 1)],
                                    scopy[:, 128 * k:128 * (k + 1)],
                                    ident_s)
            trans = tl.tile([128, 512], F32, tag="transs")
            cp_eng(trans[:], trans_ps[:])
            s1_t = tl.tile([128, UMAX, 4], F32, tag="s1")
            s1 = s1_t[:, :ucnt, :]
            nc.vector.tensor_tensor(out=s1,
                                    in0=_stats_ap(trans, ucnt, 0, 0, 2),
                                    in1=_stats_ap(trans, ucnt, 1, 0, 2),
                                    op=OP.add)
            sw_t = tl.tile([128, UMAX, 4], F32, tag="sw")
            sw = sw_t[:, :ucnt, :]
            nc.vector.tensor_tensor(out=sw,
                                    in0=_stats_ap(trans, ucnt, 0, 1, 2),
                                    in1=_stats_ap(trans, ucnt, 1, 1, 2),
                                    op=OP.add)
            s2_t = tl.tile([128, UMAX, 4], F32, tag="s2")
            s2 = s2_t[:, :ucnt, :]
            nc.vector.tensor_tensor(out=s2,
                                    in0=_stats_ap(trans, ucnt, 2, 0, 1),
                                    in1=_stats_ap(trans, ucnt, 3, 0, 1),
                                    op=OP.add)
            msq_t = tl.tile([128, UMAX, 4], F32, tag="msq")
            msq = msq_t[:, :ucnt, :]
            nc.vector.scalar_tensor_tensor(out=msq, in0=s1,
                                           scalar=1.0 / (HID * HID),
                                           in1=s1, op0=OP.mult, op1=OP.mult)
            var_t = tl.tile([128, UMAX, 4], F32, tag="var")
            var = var_t[:, :ucnt, :]
            nc.vector.scalar_tensor_tensor(out=var, in0=s2, scalar=1.0 / HID,
                                           in1=msq, op0=OP.mult,
                                           op1=OP.subtract)
            # rstd = rsqrt(var) via quake seed + Newton on gpsimd (keeps the
            # act engine on a single Exp/Tanh table; eps=1e-5 is ~1e-4 of
            # var and is dropped).
            ne = nc.gpsimd if NEWTON_ENGINE == "gpsimd" else nc.vector
            sh_t = tl.tile([128, UMAX, 4], I32, tag="sh")
            sh = sh_t[:, :ucnt, :]
            ne.tensor_scalar(out=sh, in0=var.bitcast(I32), scalar1=1,
                             scalar2=None, op0=OP.logical_shift_right)
            y_t = tl.tile([128, UMAX, 4], I32, tag="y0")
            y_i = y_t[:, :ucnt, :]
            ne.tensor_scalar(out=y_i, in0=sh, scalar1=-1,
                             scalar2=QUAKE, op0=OP.mult, op1=OP.add)
            y = y_i.bitcast(F32)
            for it in range(NEWTON_ITERS):
                p_t = tl.tile([128, UMAX, 4], F32, tag=f"nw_p{it}")
                p = p_t[:, :ucnt, :]
                ne.tensor_tensor(out=p, in0=y, in1=y, op=OP.mult)
                q_t = tl.tile([128, UMAX, 4], F32, tag=f"nw_q{it}")
                q = q_t[:, :ucnt, :]
                ne.tensor_tensor(out=q, in0=var, in1=p, op=OP.mult)
                r_t = tl.tile([128, UMAX, 4], F32, tag=f"nw_r{it}")
                r = r_t[:, :ucnt, :]
                ne.tensor_scalar(out=r, in0=q, scalar1=-0.5,
                                 scalar2=1.5, op0=OP.mult, op1=OP.add)
                y2_t = tl.tile([128, UMAX, 4], F32, tag=f"nw_y{it}")
                y2 = y2_t[:, :ucnt, :]
                ne.tensor_tensor(out=y2, in0=y, in1=r, op=OP.mult)
                y = y2
            rstd = y
            n2_t = tl.tile([128, UMAX, 4], F32, tag="n2")
            n2 = n2_t[:, :ucnt, :]
            nc.vector.scalar_tensor_tensor(out=n2, in0=s1,
                                           scalar=s_w2 / HID,
                                           in1=sw, op0=OP.mult,
                                           op1=OP.subtract)
            n3_t = tl.tile([128, UMAX, 4], F32, tag="n3")
            n3 = n3_t[:, :ucnt, :]
            nc.vector.tensor_tensor(out=n3, in0=n2, in1=rstd, op=OP.mult)
            # gate = sigmoid(alpha) = 0.5 + 0.5*tanh(alpha/2), alpha = -n3+c_a
            th_t = tl.tile([128, UMAX, 4], F32, tag="th")
            th = th_t[:, :ucnt, :]
            nc.scalar.activation(th, n3, AT.Tanh, scale=m05_s[:],
                                 bias=hca_s[:])
            gate_t = tl.tile([128, UMAX, 4], F32, tag="gate")
            gate = gate_t[:, :ucnt, :]
            nc.vector.tensor_scalar(out=gate, in0=th, scalar1=0.5,
                                    scalar2=0.5, op0=OP.mult, op1=OP.add)

            xb_t = gp.tile([128, UMAX, 4, CPN], F32, tag="xb")
            xb = xb_t[:, :ucnt, :, :]
            nc.sync.dma_start(out=xb, in_=_slab_ap(xn_d, g, ucnt))
            gb = bass.AP(tensor=gate.tensor, offset=gate.offset,
                         ap=list(gate.ap) + [[0, CPN]])
            og_t = gp.tile([128, UMAX, 4, CPN], F32, tag="og")
            og = og_t[:, :ucnt, :, :]
            last = ucnt < UMAX  # the short final group sits on the tail path
            if GATE_ENGINE == "vector" or last:
                nc.vector.tensor_tensor(out=og, in0=xb, in1=gb, op=OP.mult)
            else:
                nc.gpsimd.tensor_tensor(out=og, in0=xb, in1=gb, op=OP.mult)
            nc.sync.dma_start(out=_slab_ap(out_d, g, ucnt), in_=og)

        fin_q = []

        def emit_stats(t):
            nonlocal stats_ps
            u = t % UMAX
            if u == 0:
                stats_ps = sp.tile([128, 512], F32, tag="sps")
            start = u == 0
            stop = u == UMAX - 1 or t == NT - 1
            mm = [(0, sa_s[:, 30 - 2 * u:62 - 2 * u], vs[(t, 0)]),
                  (1, sb_s[:, 30 - 2 * u:62 - 2 * u], vs[(t, 1)]),
                  (2, sq_s[:, 31 - u:63 - u], sqs[(t, 0)]),
                  (3, sq_s[:, 31 - u:63 - u], sqs[(t, 1)])]
            for jj, lhs, rhs in mm:
                nc.tensor.matmul(stats_ps[32 * jj:32 * (jj + 1), :],
                                 lhs, rhs, start=start, stop=stop,
                                 tile_position=(0, 32 * jj),
                                 skip_group_check=True)
            del vs[(t, 0)], vs[(t, 1)], sqs[(t, 0)], sqs[(t, 1)]
            if stop:
                if FIN_DEFER:
                    # defer the finalize one slot so its scopy/transposes
                    # never head-of-line block the group's closing matmuls
                    fin_q.append((t // UMAX, u + 1, stats_ps))
                else:
                    finalize_group(t // UMAX, u + 1)

        pending = []
        n_st = (NT + 1) // 2
        for st in range(n_st):
            # 1) stats matmuls that are due (issued STAT_LAG slots ago)
            while len(pending) > 2 * STAT_LAG:
                emit_stats(pending.pop(0))
            # 2) deferred group finalize (scopy ran against idle engines)
            while fin_q:
                g, ucnt, sps = fin_q.pop(0)
                stats_ps = sps
                finalize_group(g, ucnt)

            tiles = [t for t in (2 * st, 2 * st + 1) if t < NT]
            nt_here = len(tiles)
            nf = nt_here * NTILE
            if st == 0:
                xt_t = w1a_t
            else:
                xt_t = xt_p.tile([CPN + 1, 2 * NTILE], F32R, tag="xt")
                t0 = tiles[0]
                nc.sync.dma_start(
                    out=xt_t[:, :nf],
                    in_=xt_d[:, HID + t0 * NTILE:HID + (t0 + nt_here) * NTILE])
            for half, w_sl in ((0, w1a_s[:, 0:128]), (1, w1a_s[:, 128:256])):
                z = zp.tile([128, 2 * NTILE], F32, tag="z")
                # one matmul per 512-col tile: the output must stay within a
                # single PSUM bank (512 fp32)
                for i in range(nt_here):
                    nc.tensor.matmul(z[:, i * NTILE:(i + 1) * NTILE], w_sl,
                                     xt_t[:, i * NTILE:(i + 1) * NTILE],
                                     start=True, stop=True)
                zv = z[:, :nf]
                e_t = mid.tile([128, 2 * NTILE], F16, tag="e")
                e = e_t[:, :nf]
                nc.scalar.activation(e, zv, AT.Exp, bias=neg1_s[:])
                v_t = mid.tile([128, 2 * NTILE], F16, tag=f"v{half}")
                v = v_t[:, :nf]
                vidx = 2 * st + half
                nc.vector.scalar_tensor_tensor(out=v, in0=e, scalar=1.0,
                                               in1=zv, op0=OP.min,
                                               op1=OP.max)
                sq_t = mid.tile([128, 2 * NTILE], F16, tag=f"sq{half}")
                sq = sq_t[:, :nf]
                eng = SQ_PATTERN[vidx % len(SQ_PATTERN)]
                if eng == "P":
                    nc.gpsimd.tensor_tensor(out=sq, in0=v, in1=v, op=OP.mult)
                elif eng == "A":
                    nc.scalar.activation(sq, v, AT.Square)
                else:
                    nc.vector.tensor_tensor(out=sq, in0=v, in1=v, op=OP.mult)
                for i, t in enumerate(tiles):
                    vs[(t, half)] = v[:, i * NTILE:(i + 1) * NTILE]
                    sqs[(t, half)] = sq[:, i * NTILE:(i + 1) * NTILE]

            if st == 0:
                nc.sync.dma_start(out=blob_s[:], in_=blob_d[:])
            pending.extend(tiles)
        for t in pending:
            emit_stats(t)
        while fin_q:
            g, ucnt, sps = fin_q.pop(0)
            stats_ps = sps
            finalize_group(g, ucnt)

    nc.compile()
    return nc


def _prep_params(W1, b1, gamma, beta, W2, b2):
    w1a = np.concatenate([W1, (b1 + 1.0)[None, :]], axis=0).astype(np.float32)
    w2p = (W2 * gamma).astype(np.float32)
    s_w2 = float(w2p.sum())
    c_a = float((beta * W2).sum() + b2)
    sst = np.zeros((3, 128, 64), np.float16)
    sst[0, :, 30] = 1.0
    sst[0, :, 31] = w2p[:128]
    sst[1, :, 30] = 1.0
    sst[1, :, 31] = w2p[128:]
    sst[2, :, 31] = 1.0
    blob = np.zeros((128, 224), np.float32)
    blob[:, 0:32] = sst[0].view(np.float32)
    blob[:, 32:64] = sst[1].view(np.float32)
    blob[:, 64:96] = sst[2].view(np.float32)
    blob[:, 96:224] = np.eye(128, dtype=np.float32)
    return w1a, blob, w2p, s_w2, c_a


def kernel(x, W1, b1, gamma, beta, W2, b2):
    x = np.asarray(x, np.float32)
    w1a, blob, w2p, s_w2, c_a = _prep_params(
        np.asarray(W1, np.float32), np.asarray(b1, np.float32),
        np.asarray(gamma, np.float32), np.asarray(beta, np.float32),
        np.asarray(W2, np.float32), np.asarray(b2, np.float32))

    nc = _build_program(W1, b1, w2p, s_w2, c_a)

    in_maps = []
    for c in range(NCORES):
        xs = x[c * BPC:(c + 1) * BPC].reshape(J, CPN)
        # xt column c = t*512 + k*128 + p holds node p*256 + t*4 + k so that
        # finalize-group gates land on contiguous per-partition j-slabs.
        A = xs.reshape(128, 64, 4, CPN)
        xt = np.empty((CPN + 1, J + HID), np.float32)
        perm = A.transpose(3, 1, 2, 0).reshape(CPN, J)
        xt[:CPN, :1024] = perm[:, :1024]
        xt[CPN, :1024] = 1.0
        xt[:, 1024:1024 + HID] = w1a
        xt[:CPN, 1024 + HID:] = perm[:, 1024:]
        xt[CPN, 1024 + HID:] = 1.0
        in_maps.append({"xt": np.ascontiguousarray(xt),
                        "xn": np.ascontiguousarray(xs),
                        "blob": blob})

    trace = bool(int(os.environ.get("BASS_KERNEL_TRACE", "0")))
    if not trace:
        # run_bass_kernel_spmd also honors BASS_TRACE from the ambient env;
        # the NTFF trace path crashes under this axon build (no antenv), so
        # pin tracing off unless explicitly requested here
        os.environ["BASS_NEVER_TRACE"] = "1"
    res = run_bass_kernel_spmd(nc, in_maps, list(range(NCORES)), trace=trace)
    if trace:
        kernel.last_results = res
    outs = [res.results[c]["out"].reshape(BPC, N_NODES * CPN)
            for c in range(NCORES)]
    return np.concatenate(outs, axis=0)
